# revision 7
# baseline (speedup 1.0000x reference)
"""Trainium2 Bass kernel for GQA causal attention (nn_Attention_89816356094768).

Math (per reference):
  q = x @ wq.T + bq ; k = x @ wk.T + bk ; v = x @ wv.T + bv
  RoPE on q, k; S = q @ k.T * D**-0.5 with causal mask; P = softmax(S)
  out = (P @ v) reassembled over heads @ wo.T

Sharding: tensor-parallel over heads across 8 cores. Core c owns q heads
(2c, 2c+1) and kv head c//4. Each core computes its two heads' attention and
a row-parallel partial of the output projection; the host sums the 8 partials.

On-device layout is fully transposed ("feature on partitions"): the host
pre-transposes x, the weights and the RoPE tables so every matmul contraction
dim lands on the 128 SBUF partitions with zero on-device transposes (except
v, which needs [s, d] layout for the P@v matmul and is PE-transposed).

Softmax is computed in S^T layout (scores [s, t], softmax over partitions):
exp without max-subtraction (logits are O(1) for the reference input
distribution), row sums via an all-ones-matrix matmul accumulated in PSUM,
normalization fused into the PSUM->SBUF copy of the P@v result.
"""

import numpy as np
import ml_dtypes
from contextlib import ExitStack

from concourse import bacc, tile, mybir
from concourse.bass_utils import run_bass_kernel_spmd

NQ, NKV, D = 16, 2, 128
HID = 2048
T = 4096
SCALE = D ** -0.5
NCORES = 8
HPC = NQ // NCORES          # q heads per core
P = 128                     # partitions
TS = 512                    # t-slice width (matmul moving free dim)
NT = T // P                 # 32 t tiles
NSL = T // TS               # 8 t slices
HO = HID // P               # 16 hidden k-tiles
BF16 = mybir.dt.bfloat16
F32 = mybir.dt.float32
AF = mybir.ActivationFunctionType
NPBF16 = ml_dtypes.bfloat16

_CACHE = {}


def _emit(nc, io, o_dram):
    with ExitStack() as top:
        tc = top.enter_context(tile.TileContext(nc))
        const = top.enter_context(tc.tile_pool(name="const", bufs=1))
        persist = top.enter_context(tc.tile_pool(name="persist", bufs=1))

        def cload(name, shape, dt, eng=None):
            t = const.tile(shape, dt, tag=name)
            (eng or nc.sync).dma_start(t[:], io[name][:])
            return t

        # Load order matters: the first projection matmuls need only wq/wk/wv
        # and the first x slice; defer the big phase-B constants so the PE can
        # start ~18us earlier.
        wq = cload("wqt", [P, HO, HPC * D], BF16)
        wk = cload("wkt", [P, HO, D], BF16)
        wv = cload("wvt", [P, HO, D], BF16)
        bq = cload("bq", [P, HPC], F32)
        bk = cload("bk", [P, 1], F32)
        bv = cload("bv", [P, 1], F32)
        rot = cload("rot", [P, P], BF16)
        iden = cload("iden", [P, P], BF16)
        cosT = cload("cost", [P, T], BF16, eng=nc.gpsimd)
        sinT = cload("sint", [P, T], BF16, eng=nc.gpsimd)
        ones = cload("ones", [P, P], BF16, eng=nc.gpsimd)
        maskA = cload("maska", [P, 2 * TS], BF16, eng=nc.gpsimd)
        maskB = cload("maskb", [P, 2 * TS], BF16, eng=nc.gpsimd)
        wo = cload("wot", [P, HPC, HID], BF16, eng=nc.gpsimd)

        qT = persist.tile([P, HPC, T], BF16, tag="qT")     # [d, h, t]
        kT = persist.tile([P, T], BF16, tag="kT")          # [d, s]
        vN = persist.tile([P, NT, P], BF16, tag="vN")      # [s_in, s_tile, d]
        aoT = persist.tile([P, HPC, T], BF16, tag="aoT")   # [d, h, t]

        # ---- Phase A: q/k/v projections (transposed), RoPE, v transpose ----
        with ExitStack() as pa:
            xs_pool = pa.enter_context(tc.tile_pool(name="xs", bufs=2))
            ppsum = pa.enter_context(tc.tile_pool(name="ppsum", bufs=3, space="PSUM"))
            rpsum = pa.enter_context(tc.tile_pool(name="rpsum", bufs=2, space="PSUM"))
            vpsum = pa.enter_context(tc.tile_pool(name="vpsum", bufs=2, space="PSUM"))
            rtmp = pa.enter_context(tc.tile_pool(name="rtmp", bufs=3))

            for sl in range(NSL):
                tsl = slice(sl * TS, (sl + 1) * TS)
                xt = xs_pool.tile([P, HO, TS], BF16, tag="xt")
                nc.sync.dma_start(xt[:], io["xtt"][sl])

                # (weight AP [P, HO, 128], bias [P, 1], kind, head idx)
                jobs = [(wq[:, :, h * D:(h + 1) * D], bq[:, h:h + 1], "q", h)
                        for h in range(HPC)]
                jobs.append((wk, bk, "k", 0))
                jobs.append((wv, bv, "v", 0))

                for w_ap, b_ap, kind, h in jobs:
                    pl = ppsum.tile([P, TS], F32, tag="plin")
                    for ho in range(HO):
                        nc.tensor.matmul(pl[:], w_ap[:, ho, :], xt[:, ho, :],
                                         start=(ho == 0), stop=(ho == HO - 1))
                    lin = rtmp.tile([P, TS], BF16, tag="lin")
                    nc.scalar.activation(lin[:], pl[:], AF.Identity, bias=b_ap)
                    if kind in ("q", "k"):
                        rp = rpsum.tile([P, TS], F32, tag="rp")
                        nc.tensor.matmul(rp[:], rot[:], lin[:], start=True, stop=True)
                        tsin = rtmp.tile([P, TS], F32, tag="tsin")
                        nc.vector.tensor_mul(tsin[:], rp[:], sinT[:, tsl])
                        tcos = rtmp.tile([P, TS], F32, tag="tcos")
                        nc.vector.tensor_mul(tcos[:], lin[:], cosT[:, tsl])
                        dst = qT[:, h, tsl] if kind == "q" else kT[:, tsl]
                        nc.vector.tensor_add(dst, tsin[:], tcos[:])
                    else:
                        for tt in range(TS // P):
                            vp = vpsum.tile([P, P], BF16, tag="vtp")
                            nc.tensor.transpose(vp[:], lin[:, tt * P:(tt + 1) * P],
                                                iden[:])
                            nc.vector.tensor_copy(vN[:, sl * (TS // P) + tt, :], vp[:])

        # ---- Phase B + C: attention (S^T layout flash) + output projection ----
        with ExitStack() as pb:
            stp = pb.enter_context(tc.tile_pool(name="stp", bufs=2, space="PSUM"))
            avp = pb.enter_context(tc.tile_pool(name="avp", bufs=1, space="PSUM"))
            rsp = pb.enter_context(tc.tile_pool(name="rsp", bufs=1, space="PSUM"))
            opp = pb.enter_context(tc.tile_pool(name="opp", bufs=2, space="PSUM"))
            ptp = pb.enter_context(tc.tile_pool(name="ptp", bufs=3))
            nstage = pb.enter_context(tc.tile_pool(name="nstage", bufs=2))
            ostage = pb.enter_context(tc.tile_pool(name="ostage", bufs=2))

            for sl in range(NSL):
                tsl = slice(sl * TS, (sl + 1) * TS)
                n_s = 4 * sl + 4          # causal s tiles for this slice
                ng = n_s // 2
                for h in range(HPC):
                    av = avp.tile([P, TS], F32, tag="av")
                    rs = rsp.tile([P, TS], F32, tag="rs")
                    for g in range(ng):
                        st = stp.tile([P, 2 * TS], F32, tag="st")
                        for i in range(2):
                            s_tile = 2 * g + i
                            nc.tensor.matmul(
                                st[:, i * TS:(i + 1) * TS],
                                kT[:, s_tile * P:(s_tile + 1) * P],
                                qT[:, h, tsl], start=True, stop=True)
                        pt = ptp.tile([P, 2 * TS], BF16, tag="pt")
                        nc.scalar.activation(pt[:], st[:], AF.Exp, scale=SCALE)
                        if g == ng - 2:
                            nc.vector.tensor_mul(pt[:], pt[:], maskA[:])
                        elif g == ng - 1:
                            nc.vector.tensor_mul(pt[:], pt[:], maskB[:])
                        for i in range(2):
                            s_tile = 2 * g + i
                            first = s_tile == 0
                            last = s_tile == n_s - 1
                            seg = pt[:, i * TS:(i + 1) * TS]
                            nc.tensor.matmul(av[:], vN[:, s_tile, :], seg,
                                             start=first, stop=last)
                            nc.tensor.matmul(rs[:], ones[:], seg,
                                             start=first, stop=last)
                    rec = nstage.tile([P, TS], F32, tag="rec")
                    nc.vector.reciprocal(rec[:], rs[:])
                    nc.vector.tensor_mul(aoT[:, h, tsl], av[:], rec[:])

                # output projection for this slice's 4 row blocks
                for tt4 in range(4):
                    t_tile = 4 * sl + tt4
                    trow = slice(t_tile * P, (t_tile + 1) * P)
                    ot = ostage.tile([P, HID], F32, tag="ot")
                    for upair in range(2):
                        ops = [opp.tile([P, TS], F32, tag="op", name=f"op{ui}")
                               for ui in range(2)]
                        for h in range(HPC):
                            lhs = aoT[:, h, trow]
                            for ui in range(2):
                                u0 = (upair * 2 + ui) * TS
                                nc.tensor.matmul(ops[ui][:], lhs,
                                                 wo[:, h, u0:u0 + TS],
                                                 start=(h == 0), stop=(h == HPC - 1))
                        for ui in range(2):
                            u0 = (upair * 2 + ui) * TS
                            nc.vector.tensor_copy(ot[:, u0:u0 + TS], ops[ui][:])
                    nc.sync.dma_start(o_dram[trow, :], ot[:])


def _build_nc():
    nc = bacc.Bacc("TRN2", target_bir_lowering=False, debug=False,
                   enable_asserts=False, num_devices=NCORES)
    io = {}

    def din(name, shape, dt):
        io[name] = nc.dram_tensor(name, shape, dt, kind="ExternalInput").ap()

    din("xtt", [NSL, P, HO, TS], BF16)       # x^T pre-tiled per slice
    din("wqt", [P, HO, HPC * D], BF16)
    din("wkt", [P, HO, D], BF16)
    din("wvt", [P, HO, D], BF16)
    din("wot", [P, HPC, HID], BF16)
    din("cost", [P, T], BF16)
    din("sint", [P, T], BF16)
    din("rot", [P, P], BF16)
    din("iden", [P, P], BF16)
    din("ones", [P, P], BF16)
    din("maska", [P, 2 * TS], BF16)
    din("maskb", [P, 2 * TS], BF16)
    din("bq", [P, HPC], F32)
    din("bk", [P, 1], F32)
    din("bv", [P, 1], F32)
    o = nc.dram_tensor("o_part", [T, HID], F32, kind="ExternalOutput").ap()
    _emit(nc, io, o)
    nc.compile()
    return nc


def _get_nc():
    if "nc" not in _CACHE:
        _CACHE["nc"] = _build_nc()
    return _CACHE["nc"]


def _consts():
    if "consts" in _CACHE:
        return _CACHE["consts"]
    # rotate_half as a matmul on q^T: out[d,t] = sum_e R[e,d] q^T[e,t]
    R = np.zeros((P, P), np.float32)
    for e in range(64):
        R[e, e + 64] = 1.0          # d >= 64 takes +q[d-64]
    for e in range(64, P):
        R[e, e - 64] = -1.0         # d < 64 takes -q[d+64]
    iden = np.eye(P, dtype=np.float32)
    onesm = np.ones((P, P), np.float32)
    s_in = np.arange(P)[:, None]
    col = np.arange(TS)[None, :]
    masks = [(col >= r * P + s_in).astype(np.float32) for r in range(4)]
    maskA = np.concatenate([masks[0], masks[1]], axis=1)
    maskB = np.concatenate([masks[2], masks[3]], axis=1)
    _CACHE["consts"] = tuple(a.astype(NPBF16) for a in (R, iden, onesm, maskA, maskB))
    return _CACHE["consts"]


def kernel(x, cos, sin, wq, bq, wk, bk, wv, bv, wo):
    x = np.asarray(x, dtype=np.float32)
    cos = np.asarray(cos, dtype=np.float32)
    sin = np.asarray(sin, dtype=np.float32)
    wq = np.asarray(wq, dtype=np.float32)
    bq = np.asarray(bq, dtype=np.float32)
    wk = np.asarray(wk, dtype=np.float32)
    bk = np.asarray(bk, dtype=np.float32)
    wv = np.asarray(wv, dtype=np.float32)
    bv = np.asarray(bv, dtype=np.float32)
    wo = np.asarray(wo, dtype=np.float32)

    nc = _get_nc()
    R, iden, onesm, maskA, maskB = _consts()

    # x^T tiled: xtt[sl, p, ho, c] = x[0, sl*TS + c, ho*P + p]
    xT = np.ascontiguousarray(x[0].T).astype(NPBF16)              # [HID, T]
    xtt = np.ascontiguousarray(
        xT.reshape(HO, P, NSL, TS).transpose(2, 1, 0, 3))         # [NSL,P,HO,TS]
    cosT = np.ascontiguousarray(cos.T).astype(NPBF16)             # [P, T]
    sinT = np.ascontiguousarray(sin.T).astype(NPBF16)

    def wtile(wslice):  # [J, HID] -> [P, HO, J] with h = ho*P + p
        J = wslice.shape[0]
        return np.ascontiguousarray(
            wslice.T.reshape(HO, P, J).transpose(1, 0, 2)).astype(NPBF16)

    in_maps = []
    for c in range(NCORES):
        j0 = c * HPC * D
        kvh = c // (NCORES // NKV)
        wqt = wtile(wq[j0:j0 + HPC * D])                          # [P, HO, 256]
        wkt = wtile(wk[kvh * D:(kvh + 1) * D])                    # [P, HO, 128]
        wvt = wtile(wv[kvh * D:(kvh + 1) * D])
        # woT: [P, HPC, HID] with j = h*P + p
        wot = np.ascontiguousarray(
            wo[:, j0:j0 + HPC * D].T.reshape(HPC, P, HID)
            .transpose(1, 0, 2)).astype(NPBF16)
        bqt = np.ascontiguousarray(
            bq[j0:j0 + HPC * D].reshape(HPC, P).T).astype(np.float32)
        bkt = bk[kvh * D:(kvh + 1) * D].reshape(P, 1).astype(np.float32)
        bvt = bv[kvh * D:(kvh + 1) * D].reshape(P, 1).astype(np.float32)
        in_maps.append({
            "xtt": xtt, "wqt": wqt, "wkt": wkt, "wvt": wvt, "wot": wot,
            "cost": cosT, "sint": sinT, "rot": R, "iden": iden, "ones": onesm,
            "maska": maskA, "maskb": maskB,
            "bq": bqt, "bk": bkt, "bv": bvt,
        })

    res = run_bass_kernel_spmd(nc, in_maps, list(range(NCORES)))
    out = np.zeros((T, HID), np.float32)
    for c in range(NCORES):
        out += res.results[c]["o_part"]
    return out.reshape(1, T, HID)


# revision 18
# speedup vs baseline: 1.0733x; 1.0733x over previous
"""Trainium2 Bass kernel for GQA causal attention (nn_Attention_89816356094768).

Math (per reference):
  q = x @ wq.T + bq ; k = x @ wk.T + bk ; v = x @ wv.T + bv
  RoPE on q, k; S = q @ k.T * D**-0.5 with causal mask; P = softmax(S)
  out = (P @ v) reassembled over heads @ wo.T

Sharding: tensor-parallel over heads across 8 cores. Core c owns q heads
(2c, 2c+1) and kv head c//4. Each core computes its two heads' attention and
a row-parallel partial of the output projection; the host sums the 8 partials.

On-device layout is fully transposed ("feature on partitions"): the host
pre-transposes x, the weights and the RoPE tables so every matmul contraction
dim lands on the 128 SBUF partitions with zero on-device transposes (except
v, which needs [s, d] layout for the P@v matmul and is PE-transposed).

Softmax is computed in S^T layout (scores [s, t], softmax over partitions):
exp without max-subtraction (logits are O(1) for the reference input
distribution), row sums via an all-ones-matrix matmul accumulated in PSUM,
normalization fused into the PSUM->SBUF copy of the P@v result.
"""

import numpy as np
import ml_dtypes
from contextlib import ExitStack

from concourse import bacc, tile, mybir
from concourse.bass_utils import run_bass_kernel_spmd

NQ, NKV, D = 16, 2, 128
HID = 2048
T = 4096
SCALE = D ** -0.5
NCORES = 8
HPC = NQ // NCORES          # q heads per core
P = 128                     # partitions
TS = 512                    # t-slice width (matmul moving free dim)
NT = T // P                 # 32 t tiles
NSL = T // TS               # 8 t slices
HO = HID // P               # 16 hidden k-tiles
BF16 = mybir.dt.bfloat16
F32 = mybir.dt.float32
AF = mybir.ActivationFunctionType
NPBF16 = ml_dtypes.bfloat16

_CACHE = {}


def _emit(nc, io, o_dram):
    with ExitStack() as top:
        tc = top.enter_context(tile.TileContext(nc))
        const = top.enter_context(tc.tile_pool(name="const", bufs=1))
        persist = top.enter_context(tc.tile_pool(name="persist", bufs=1))

        def cload(name, shape, dt, eng=None):
            t = const.tile(shape, dt, tag=name)
            (eng or nc.sync).dma_start(t[:], io[name][:])
            return t

        # Load order matters: the first projection matmuls need only wq/wk/wv
        # and the first x slice; defer the big phase-B constants so the PE can
        # start ~18us earlier.
        xs_pool = top.enter_context(tc.tile_pool(name="xs", bufs=2))

        # The first projection chain consumes wq and x-slice-0 h-tile by
        # h-tile, so interleave their loads in 4 chunks each and put
        # everything else behind them; phase-B-only constants trail last.
        wq = const.tile([P, HO, HPC * D], BF16, tag="wqt")
        xt0 = xs_pool.tile([P, HO, TS], BF16, tag="xt", name="xt0")
        for ch in range(4):
            hs = slice(4 * ch, 4 * (ch + 1))
            nc.sync.dma_start(wq[:, hs, :], io["wqt"][:, hs, :])
            nc.sync.dma_start(xt0[:, hs, :], io["xtt"][0, :, hs, :])
        bq = cload("bq", [P, HPC], F32)
        rot = cload("rot", [P, P], BF16)
        wk = cload("wkt", [P, HO, D], BF16)
        wv = cload("wvt", [P, HO, D], BF16)
        bk = cload("bk", [P, 1], F32)
        bv = cload("bv", [P, 1], F32)
        iden = cload("iden", [P, P], BF16)
        cosT = cload("cost", [P, T], BF16)
        sinT = cload("sint", [P, T], BF16)
        ones = cload("ones", [P, P], BF16, eng=nc.gpsimd)
        tri = cload("tri", [P, P], BF16, eng=nc.gpsimd)
        wo = cload("wot", [P, HPC, HID], BF16, eng=nc.gpsimd)

        qT = persist.tile([P, HPC, T], BF16, tag="qT")     # [d, h, t]
        kT = persist.tile([P, T], BF16, tag="kT")          # [d, s]
        vN = persist.tile([P, NT, P], BF16, tag="vN")      # [s_in, s_tile, d]
        aoT = persist.tile([P, HPC, T], BF16, tag="aoT")   # [d, h, t]

        # ---- Phase A: q/k/v projections (transposed), RoPE, v transpose ----
        with ExitStack() as pa:
            ppsum = pa.enter_context(tc.tile_pool(name="ppsum", bufs=4, space="PSUM"))
            rpsum = pa.enter_context(tc.tile_pool(name="rpsum", bufs=2, space="PSUM"))
            vpsum = pa.enter_context(tc.tile_pool(name="vpsum", bufs=2, space="PSUM"))
            rtmp = pa.enter_context(tc.tile_pool(name="rtmp", bufs=4))

            for sl in range(NSL):
                tsl = slice(sl * TS, (sl + 1) * TS)
                if sl == 0:
                    xt = xt0
                else:
                    xt = xs_pool.tile([P, HO, TS], BF16, tag="xt")
                    nc.sync.dma_start(xt[:], io["xtt"][sl])

                # (weight AP [P, HO, 128], bias [P, 1], kind, head idx)
                jobs = [(wq[:, :, h * D:(h + 1) * D], bq[:, h:h + 1], "q", h)
                        for h in range(HPC)]
                jobs.append((wk, bk, "k", 0))
                jobs.append((wv, bv, "v", 0))

                for w_ap, b_ap, kind, h in jobs:
                    pl = ppsum.tile([P, TS], F32, tag="plin")
                    for ho in range(HO):
                        nc.tensor.matmul(pl[:], w_ap[:, ho, :], xt[:, ho, :],
                                         start=(ho == 0), stop=(ho == HO - 1))
                    lin = rtmp.tile([P, TS], BF16, tag="lin")
                    nc.scalar.activation(lin[:], pl[:], AF.Identity, bias=b_ap)
                    if kind in ("q", "k"):
                        rp = rpsum.tile([P, TS], F32, tag="rp")
                        nc.tensor.matmul(rp[:], rot[:], lin[:], start=True, stop=True)
                        tsin = rtmp.tile([P, TS], F32, tag="tsin")
                        nc.vector.tensor_mul(tsin[:], rp[:], sinT[:, tsl])
                        tcos = rtmp.tile([P, TS], F32, tag="tcos")
                        nc.vector.tensor_mul(tcos[:], lin[:], cosT[:, tsl])
                        dst = qT[:, h, tsl] if kind == "q" else kT[:, tsl]
                        nc.vector.tensor_add(dst, tsin[:], tcos[:])
                    else:
                        for tt in range(TS // P):
                            vp = vpsum.tile([P, P], BF16, tag="vtp")
                            nc.tensor.transpose(vp[:], lin[:, tt * P:(tt + 1) * P],
                                                iden[:])
                            nc.vector.tensor_copy(vN[:, sl * (TS // P) + tt, :], vp[:])

        # ---- Phase B + C: attention (S^T layout flash) + output projection ----
        with ExitStack() as pb:
            # PSUM budget (8 banks): stp 2x[P,1024]=4, avp 1, rsp 1, opp 2
            stp = pb.enter_context(tc.tile_pool(name="stp", bufs=2, space="PSUM"))
            avp = pb.enter_context(tc.tile_pool(name="avp", bufs=2, space="PSUM"))
            rsp = pb.enter_context(tc.tile_pool(name="rsp", bufs=1, space="PSUM"))
            opp = pb.enter_context(tc.tile_pool(name="opp", bufs=1, space="PSUM"))
            ptp = pb.enter_context(tc.tile_pool(name="ptp", bufs=4))
            nstage = pb.enter_context(tc.tile_pool(name="nstage", bufs=2))
            ostage = pb.enter_context(tc.tile_pool(name="ostage", bufs=2))

            for sl in range(NSL):
                tsl = slice(sl * TS, (sl + 1) * TS)
                n_s = 4 * sl + 4          # causal s tiles for this slice
                ng = n_s // 2
                for h in range(HPC):
                    av = avp.tile([P, TS], F32, tag="av")
                    rs = rsp.tile([P, TS], F32, tag="rs")
                    for g in range(ng):
                        st = stp.tile([P, 2 * TS], F32, tag="st")
                        pt = ptp.tile([P, 2 * TS], BF16, tag="pt")
                        # r >= 0 marks a diagonal-region s tile: its first
                        # r*P t-columns are fully masked, so skip them in the
                        # matmuls and exp, and mask only the diagonal block.
                        offs = [max(2 * g + i - 4 * sl, 0) * P for i in range(2)]
                        for i in range(2):
                            s_tile = 2 * g + i
                            off = offs[i]
                            nc.tensor.matmul(
                                st[:, i * TS + off:(i + 1) * TS],
                                kT[:, s_tile * P:(s_tile + 1) * P],
                                qT[:, h, sl * TS + off:(sl + 1) * TS],
                                start=True, stop=True)
                        if offs[1] == 0:
                            # split per segment: halves the exp latency the
                            # first P@v matmul of the group waits on
                            for i in range(2):
                                nc.scalar.activation(
                                    pt[:, i * TS:(i + 1) * TS],
                                    st[:, i * TS:(i + 1) * TS],
                                    AF.Exp, scale=SCALE)
                        else:
                            for i in range(2):
                                off = offs[i]
                                if off:
                                    nc.gpsimd.memset(pt[:, i * TS:i * TS + off], 0.0)
                                nc.scalar.activation(
                                    pt[:, i * TS + off:(i + 1) * TS],
                                    st[:, i * TS + off:(i + 1) * TS],
                                    AF.Exp, scale=SCALE)
                                c0 = i * TS + off
                                nc.vector.tensor_mul(pt[:, c0:c0 + P],
                                                     pt[:, c0:c0 + P], tri[:])
                        for i in range(2):
                            s_tile = 2 * g + i
                            off = offs[i]
                            first = s_tile == 0
                            last = s_tile == n_s - 1
                            seg = pt[:, i * TS + off:(i + 1) * TS]
                            nc.tensor.matmul(av[:, off:TS], vN[:, s_tile, :], seg,
                                             start=first, stop=last)
                            nc.tensor.matmul(rs[:, off:TS], ones[:], seg,
                                             start=first, stop=last)
                    rec = nstage.tile([P, TS], F32, tag="rec")
                    nc.vector.reciprocal(rec[:], rs[:])
                    nc.vector.tensor_mul(aoT[:, h, tsl], av[:], rec[:])

                # output projection for this slice's 4 row blocks
                for tt4 in range(4):
                    t_tile = 4 * sl + tt4
                    trow = slice(t_tile * P, (t_tile + 1) * P)
                    ot = ostage.tile([P, HID], F32, tag="ot")
                    for upair in range(2):
                        ops = [opp.tile([P, TS], F32, tag="op", name=f"op{ui}")
                               for ui in range(2)]
                        for h in range(HPC):
                            lhs = aoT[:, h, trow]
                            for ui in range(2):
                                u0 = (upair * 2 + ui) * TS
                                nc.tensor.matmul(ops[ui][:], lhs,
                                                 wo[:, h, u0:u0 + TS],
                                                 start=(h == 0), stop=(h == HPC - 1))
                        for ui in range(2):
                            u0 = (upair * 2 + ui) * TS
                            nc.vector.tensor_copy(ot[:, u0:u0 + TS], ops[ui][:])
                    nc.sync.dma_start(o_dram[trow, :], ot[:])


def _build_nc():
    nc = bacc.Bacc("TRN2", target_bir_lowering=False, debug=False,
                   enable_asserts=False, num_devices=NCORES)
    io = {}

    def din(name, shape, dt):
        io[name] = nc.dram_tensor(name, shape, dt, kind="ExternalInput").ap()

    din("xtt", [NSL, P, HO, TS], BF16)       # x^T pre-tiled per slice
    din("wqt", [P, HO, HPC * D], BF16)
    din("wkt", [P, HO, D], BF16)
    din("wvt", [P, HO, D], BF16)
    din("wot", [P, HPC, HID], BF16)
    din("cost", [P, T], BF16)
    din("sint", [P, T], BF16)
    din("rot", [P, P], BF16)
    din("iden", [P, P], BF16)
    din("ones", [P, P], BF16)
    din("tri", [P, P], BF16)
    din("bq", [P, HPC], F32)
    din("bk", [P, 1], F32)
    din("bv", [P, 1], F32)
    o = nc.dram_tensor("o_part", [T, HID], F32, kind="ExternalOutput").ap()
    _emit(nc, io, o)
    nc.compile()
    return nc


def _get_nc():
    if "nc" not in _CACHE:
        _CACHE["nc"] = _build_nc()
    return _CACHE["nc"]


def _consts():
    if "consts" in _CACHE:
        return _CACHE["consts"]
    # rotate_half as a matmul on q^T: out[d,t] = sum_e R[e,d] q^T[e,t]
    R = np.zeros((P, P), np.float32)
    for e in range(64):
        R[e, e + 64] = 1.0          # d >= 64 takes +q[d-64]
    for e in range(64, P):
        R[e, e - 64] = -1.0         # d < 64 takes -q[d+64]
    iden = np.eye(P, dtype=np.float32)
    onesm = np.ones((P, P), np.float32)
    tri = np.triu(np.ones((P, P), np.float32))
    _CACHE["consts"] = tuple(a.astype(NPBF16) for a in (R, iden, onesm, tri))
    return _CACHE["consts"]


def kernel(x, cos, sin, wq, bq, wk, bk, wv, bv, wo):
    x = np.asarray(x, dtype=np.float32)
    cos = np.asarray(cos, dtype=np.float32)
    sin = np.asarray(sin, dtype=np.float32)
    wq = np.asarray(wq, dtype=np.float32)
    bq = np.asarray(bq, dtype=np.float32)
    wk = np.asarray(wk, dtype=np.float32)
    bk = np.asarray(bk, dtype=np.float32)
    wv = np.asarray(wv, dtype=np.float32)
    bv = np.asarray(bv, dtype=np.float32)
    wo = np.asarray(wo, dtype=np.float32)

    nc = _get_nc()
    R, iden, onesm, tri = _consts()

    # x^T tiled: xtt[sl, p, ho, c] = x[0, sl*TS + c, ho*P + p]
    xT = np.ascontiguousarray(x[0].T).astype(NPBF16)              # [HID, T]
    xtt = np.ascontiguousarray(
        xT.reshape(HO, P, NSL, TS).transpose(2, 1, 0, 3))         # [NSL,P,HO,TS]
    cosT = np.ascontiguousarray(cos.T).astype(NPBF16)             # [P, T]
    sinT = np.ascontiguousarray(sin.T).astype(NPBF16)

    def wtile(wslice):  # [J, HID] -> [P, HO, J] with h = ho*P + p
        J = wslice.shape[0]
        return np.ascontiguousarray(
            wslice.T.reshape(HO, P, J).transpose(1, 0, 2)).astype(NPBF16)

    in_maps = []
    for c in range(NCORES):
        j0 = c * HPC * D
        kvh = c // (NCORES // NKV)
        wqt = wtile(wq[j0:j0 + HPC * D])                          # [P, HO, 256]
        wkt = wtile(wk[kvh * D:(kvh + 1) * D])                    # [P, HO, 128]
        wvt = wtile(wv[kvh * D:(kvh + 1) * D])
        # woT: [P, HPC, HID] with j = h*P + p
        wot = np.ascontiguousarray(
            wo[:, j0:j0 + HPC * D].T.reshape(HPC, P, HID)
            .transpose(1, 0, 2)).astype(NPBF16)
        bqt = np.ascontiguousarray(
            bq[j0:j0 + HPC * D].reshape(HPC, P).T).astype(np.float32)
        bkt = bk[kvh * D:(kvh + 1) * D].reshape(P, 1).astype(np.float32)
        bvt = bv[kvh * D:(kvh + 1) * D].reshape(P, 1).astype(np.float32)
        in_maps.append({
            "xtt": xtt, "wqt": wqt, "wkt": wkt, "wvt": wvt, "wot": wot,
            "cost": cosT, "sint": sinT, "rot": R, "iden": iden, "ones": onesm,
            "tri": tri,
            "bq": bqt, "bk": bkt, "bv": bvt,
        })

    res = run_bass_kernel_spmd(nc, in_maps, list(range(NCORES)))
    out = np.zeros((T, HID), np.float32)
    for c in range(NCORES):
        out += res.results[c]["o_part"]
    return out.reshape(1, T, HID)


# revision 26
# speedup vs baseline: 1.1046x; 1.0292x over previous
"""Trainium2 Bass kernel for GQA causal attention (nn_Attention_89816356094768).

Math (per reference):
  q = x @ wq.T + bq ; k = x @ wk.T + bk ; v = x @ wv.T + bv
  RoPE on q, k; S = q @ k.T * D**-0.5 with causal mask; P = softmax(S)
  out = (P @ v) reassembled over heads @ wo.T

Sharding: tensor-parallel over heads across 8 cores. Core c owns q heads
(2c, 2c+1) and kv head c//4. Each core computes its two heads' attention and
a row-parallel partial of the output projection; the host sums the 8 partials.

On-device layout is fully transposed ("feature on partitions"): the host
pre-transposes x, the weights and the RoPE tables so every matmul contraction
dim lands on the 128 SBUF partitions with zero on-device transposes (except
v, which needs [s, d] layout for the P@v matmul and is PE-transposed).

Softmax is computed in S^T layout (scores [s, t], softmax over partitions):
exp without max-subtraction (logits are O(1) for the reference input
distribution), row sums via an all-ones-matrix matmul accumulated in PSUM,
normalization fused into the PSUM->SBUF copy of the P@v result.
"""

import numpy as np
import ml_dtypes
from contextlib import ExitStack

from concourse import bacc, tile, mybir
from concourse.bass_utils import run_bass_kernel_spmd

NQ, NKV, D = 16, 2, 128
HID = 2048
T = 4096
SCALE = D ** -0.5
NCORES = 8
HPC = NQ // NCORES          # q heads per core
P = 128                     # partitions
TS = 512                    # t-slice width (matmul moving free dim)
NT = T // P                 # 32 t tiles
NSL = T // TS               # 8 t slices
HO = HID // P               # 16 hidden k-tiles
BF16 = mybir.dt.bfloat16
F32 = mybir.dt.float32
AF = mybir.ActivationFunctionType
NPBF16 = ml_dtypes.bfloat16

_CACHE = {}


def _emit(nc, io, o_dram):
    with ExitStack() as top:
        tc = top.enter_context(tile.TileContext(nc))
        const = top.enter_context(tc.tile_pool(name="const", bufs=1))
        persist = top.enter_context(tc.tile_pool(name="persist", bufs=1))

        def cload(name, shape, dt, eng=None):
            t = const.tile(shape, dt, tag=name)
            (eng or nc.sync).dma_start(t[:], io[name][:])
            return t

        # Load order matters: the first projection matmuls need only wq/wk/wv
        # and the first x slice; defer the big phase-B constants so the PE can
        # start ~18us earlier.
        xs_pool = top.enter_context(tc.tile_pool(name="xs", bufs=2))

        # The first projection chain consumes wq and x-slice-0 h-tile by
        # h-tile, so interleave their loads in 4 chunks each and put
        # everything else behind them; phase-B-only constants trail last.
        wq = const.tile([P, HO, HPC * D], BF16, tag="wqt")
        xt0 = xs_pool.tile([P, HO, TS], BF16, tag="xt", name="xt0")
        for ch in range(4):
            hs = slice(4 * ch, 4 * (ch + 1))
            nc.sync.dma_start(wq[:, hs, :], io["wqt"][:, hs, :])
            nc.sync.dma_start(xt0[:, hs, :], io["xtt"][0, :, hs, :])
        bq = cload("bq", [P, HPC], F32)
        rot = cload("rot", [P, P], BF16)
        wk = cload("wkt", [P, HO, D], BF16)
        wv = cload("wvt", [P, HO, D], BF16)
        bk = cload("bk", [P, 1], F32)
        bv = cload("bv", [P, 1], F32)
        iden = cload("iden", [P, P], BF16)
        xt1 = xs_pool.tile([P, HO, TS], BF16, tag="xt", name="xt1")
        nc.sync.dma_start(xt1[:], io["xtt"][1])
        cosT = cload("cost", [P, T], BF16)
        sinT = cload("sint", [P, T], BF16)
        ones = cload("ones", [P, P], BF16, eng=nc.gpsimd)
        tri = cload("tri", [P, P], BF16, eng=nc.gpsimd)
        wo = cload("wot", [P, HPC, HID], BF16, eng=nc.gpsimd)

        qT = persist.tile([P, HPC, T], BF16, tag="qT")     # [d, h, t]
        kT = persist.tile([P, T], BF16, tag="kT")          # [d, s]
        vN = persist.tile([P, NT, P], BF16, tag="vN")      # [s_in, s_tile, d]
        aoT = persist.tile([P, HPC, T], BF16, tag="aoT")   # [d, h, t]

        # ---- Phase A: q/k/v projections (transposed), RoPE, v transpose ----
        with ExitStack() as pa:
            ppsum = pa.enter_context(tc.tile_pool(name="ppsum", bufs=4, space="PSUM"))
            rpsum = pa.enter_context(tc.tile_pool(name="rpsum", bufs=2, space="PSUM"))
            vpsum = pa.enter_context(tc.tile_pool(name="vpsum", bufs=2, space="PSUM"))
            rtmp = pa.enter_context(tc.tile_pool(name="rtmp", bufs=4))

            for sl in range(NSL):
                tsl = slice(sl * TS, (sl + 1) * TS)
                if sl == 0:
                    xt = xt0
                elif sl == 1:
                    xt = xt1
                else:
                    xt = xs_pool.tile([P, HO, TS], BF16, tag="xt")
                    nc.sync.dma_start(xt[:], io["xtt"][sl])

                # (weight AP [P, HO, 128], bias [P, 1], kind, head idx)
                jobs = [(wq[:, :, h * D:(h + 1) * D], bq[:, h:h + 1], "q", h)
                        for h in range(HPC)]
                jobs.append((wk, bk, "k", 0))
                jobs.append((wv, bv, "v", 0))

                for w_ap, b_ap, kind, h in jobs:
                    pl = ppsum.tile([P, TS], F32, tag="plin")
                    for ho in range(HO):
                        nc.tensor.matmul(pl[:], w_ap[:, ho, :], xt[:, ho, :],
                                         start=(ho == 0), stop=(ho == HO - 1))
                    lin = rtmp.tile([P, TS], BF16, tag="lin")
                    nc.scalar.activation(lin[:], pl[:], AF.Identity, bias=b_ap)
                    if kind in ("q", "k"):
                        rp = rpsum.tile([P, TS], F32, tag="rp")
                        nc.tensor.matmul(rp[:], rot[:], lin[:], start=True, stop=True)
                        tsin = rtmp.tile([P, TS], F32, tag="tsin")
                        nc.vector.tensor_mul(tsin[:], rp[:], sinT[:, tsl])
                        tcos = rtmp.tile([P, TS], F32, tag="tcos")
                        nc.vector.tensor_mul(tcos[:], lin[:], cosT[:, tsl])
                        dst = qT[:, h, tsl] if kind == "q" else kT[:, tsl]
                        nc.vector.tensor_add(dst, tsin[:], tcos[:])
                    else:
                        for tt in range(TS // P):
                            vp = vpsum.tile([P, P], BF16, tag="vtp")
                            nc.tensor.transpose(vp[:], lin[:, tt * P:(tt + 1) * P],
                                                iden[:])
                            nc.vector.tensor_copy(vN[:, sl * (TS // P) + tt, :], vp[:])

        # ---- Phase B + C: attention (S^T layout flash) + output projection ----
        with ExitStack() as pb:
            # PSUM budget (8 banks): stp 2x[P,1024]=4, avp 1, rsp 1, opp 2
            stp = pb.enter_context(tc.tile_pool(name="stp", bufs=2, space="PSUM"))
            avp = pb.enter_context(tc.tile_pool(name="avp", bufs=2, space="PSUM"))
            rsp = pb.enter_context(tc.tile_pool(name="rsp", bufs=1, space="PSUM"))
            opp = pb.enter_context(tc.tile_pool(name="opp", bufs=1, space="PSUM"))
            ptp = pb.enter_context(tc.tile_pool(name="ptp", bufs=6))
            nstage = pb.enter_context(tc.tile_pool(name="nstage", bufs=2))
            ostage = pb.enter_context(tc.tile_pool(name="ostage", bufs=2))

            for sl in range(NSL):
                tsl = slice(sl * TS, (sl + 1) * TS)
                n_s = 4 * sl + 4          # causal s tiles for this slice
                ng = n_s // 2
                for h in range(HPC):
                    av = avp.tile([P, TS], F32, tag="av")
                    rs = rsp.tile([P, TS], F32, tag="rs")
                    for g in range(ng):
                        st = stp.tile([P, 2 * TS], F32, tag="st")
                        pt = ptp.tile([P, 2 * TS], BF16, tag="pt")
                        # r >= 0 marks a diagonal-region s tile: its first
                        # r*P t-columns are fully masked, so skip them in the
                        # matmuls and exp, and mask only the diagonal block.
                        offs = [max(2 * g + i - 4 * sl, 0) * P for i in range(2)]
                        for i in range(2):
                            s_tile = 2 * g + i
                            off = offs[i]
                            nc.tensor.matmul(
                                st[:, i * TS + off:(i + 1) * TS],
                                kT[:, s_tile * P:(s_tile + 1) * P],
                                qT[:, h, sl * TS + off:(sl + 1) * TS],
                                start=True, stop=True)
                        if offs[1] == 0:
                            nc.scalar.activation(pt[:], st[:], AF.Exp, scale=SCALE)
                        else:
                            for i in range(2):
                                off = offs[i]
                                if off:
                                    nc.gpsimd.memset(pt[:, i * TS:i * TS + off], 0.0)
                                nc.scalar.activation(
                                    pt[:, i * TS + off:(i + 1) * TS],
                                    st[:, i * TS + off:(i + 1) * TS],
                                    AF.Exp, scale=SCALE)
                                c0 = i * TS + off
                                nc.vector.tensor_mul(pt[:, c0:c0 + P],
                                                     pt[:, c0:c0 + P], tri[:])
                        for i in range(2):
                            s_tile = 2 * g + i
                            off = offs[i]
                            first = s_tile == 0
                            last = s_tile == n_s - 1
                            seg = pt[:, i * TS + off:(i + 1) * TS]
                            nc.tensor.matmul(av[:, off:TS], vN[:, s_tile, :], seg,
                                             start=first, stop=last)
                            nc.tensor.matmul(rs[:, off:TS], ones[:], seg,
                                             start=first, stop=last)
                    rec = nstage.tile([P, TS], F32, tag="rec")
                    nc.vector.reciprocal(rec[:], rs[:])
                    nc.vector.tensor_mul(aoT[:, h, tsl], av[:], rec[:])

                # output projection for this slice's 4 row blocks; the final
                # slice borrows the score-psum pool (attention is done by
                # then) to double-buffer instead of serializing on opp
                fin = sl == NSL - 1
                for tt4 in range(4):
                    t_tile = 4 * sl + tt4
                    trow = slice(t_tile * P, (t_tile + 1) * P)
                    ot = ostage.tile([P, HID], F32, tag="ot")
                    for upair in range(2):
                        if fin:
                            op2 = stp.tile([P, 2 * TS], F32, tag="st", name="op2")
                            ops = [op2[:, :TS], op2[:, TS:]]
                        else:
                            ops = [opp.tile([P, TS], F32, tag="op", name=f"op{ui}")[:]
                                   for ui in range(2)]
                        for h in range(HPC):
                            lhs = aoT[:, h, trow]
                            for ui in range(2):
                                u0 = (upair * 2 + ui) * TS
                                nc.tensor.matmul(ops[ui], lhs,
                                                 wo[:, h, u0:u0 + TS],
                                                 start=(h == 0), stop=(h == HPC - 1))
                        if fin:
                            nc.any.tensor_copy(
                                ot[:, upair * 2 * TS:(upair + 1) * 2 * TS], op2[:])
                        else:
                            for ui in range(2):
                                u0 = (upair * 2 + ui) * TS
                                nc.vector.tensor_copy(ot[:, u0:u0 + TS], ops[ui])
                    nc.sync.dma_start(o_dram[trow, :], ot[:])


def _build_nc():
    nc = bacc.Bacc("TRN2", target_bir_lowering=False, debug=False,
                   enable_asserts=False, num_devices=NCORES)
    io = {}

    def din(name, shape, dt):
        io[name] = nc.dram_tensor(name, shape, dt, kind="ExternalInput").ap()

    din("xtt", [NSL, P, HO, TS], BF16)       # x^T pre-tiled per slice
    din("wqt", [P, HO, HPC * D], BF16)
    din("wkt", [P, HO, D], BF16)
    din("wvt", [P, HO, D], BF16)
    din("wot", [P, HPC, HID], BF16)
    din("cost", [P, T], BF16)
    din("sint", [P, T], BF16)
    din("rot", [P, P], BF16)
    din("iden", [P, P], BF16)
    din("ones", [P, P], BF16)
    din("tri", [P, P], BF16)
    din("bq", [P, HPC], F32)
    din("bk", [P, 1], F32)
    din("bv", [P, 1], F32)
    o = nc.dram_tensor("o_part", [T, HID], F32, kind="ExternalOutput").ap()
    _emit(nc, io, o)
    nc.compile()
    return nc


def _get_nc():
    if "nc" not in _CACHE:
        _CACHE["nc"] = _build_nc()
    return _CACHE["nc"]


def _consts():
    if "consts" in _CACHE:
        return _CACHE["consts"]
    # rotate_half as a matmul on q^T: out[d,t] = sum_e R[e,d] q^T[e,t]
    R = np.zeros((P, P), np.float32)
    for e in range(64):
        R[e, e + 64] = 1.0          # d >= 64 takes +q[d-64]
    for e in range(64, P):
        R[e, e - 64] = -1.0         # d < 64 takes -q[d+64]
    iden = np.eye(P, dtype=np.float32)
    onesm = np.ones((P, P), np.float32)
    tri = np.triu(np.ones((P, P), np.float32))
    _CACHE["consts"] = tuple(a.astype(NPBF16) for a in (R, iden, onesm, tri))
    return _CACHE["consts"]


def kernel(x, cos, sin, wq, bq, wk, bk, wv, bv, wo):
    x = np.asarray(x, dtype=np.float32)
    cos = np.asarray(cos, dtype=np.float32)
    sin = np.asarray(sin, dtype=np.float32)
    wq = np.asarray(wq, dtype=np.float32)
    bq = np.asarray(bq, dtype=np.float32)
    wk = np.asarray(wk, dtype=np.float32)
    bk = np.asarray(bk, dtype=np.float32)
    wv = np.asarray(wv, dtype=np.float32)
    bv = np.asarray(bv, dtype=np.float32)
    wo = np.asarray(wo, dtype=np.float32)

    nc = _get_nc()
    R, iden, onesm, tri = _consts()

    # x^T tiled: xtt[sl, p, ho, c] = x[0, sl*TS + c, ho*P + p]
    xT = np.ascontiguousarray(x[0].T).astype(NPBF16)              # [HID, T]
    xtt = np.ascontiguousarray(
        xT.reshape(HO, P, NSL, TS).transpose(2, 1, 0, 3))         # [NSL,P,HO,TS]
    cosT = np.ascontiguousarray(cos.T).astype(NPBF16)             # [P, T]
    sinT = np.ascontiguousarray(sin.T).astype(NPBF16)

    def wtile(wslice):  # [J, HID] -> [P, HO, J] with h = ho*P + p
        J = wslice.shape[0]
        return np.ascontiguousarray(
            wslice.T.reshape(HO, P, J).transpose(1, 0, 2)).astype(NPBF16)

    in_maps = []
    for c in range(NCORES):
        j0 = c * HPC * D
        kvh = c // (NCORES // NKV)
        wqt = wtile(wq[j0:j0 + HPC * D])                          # [P, HO, 256]
        wkt = wtile(wk[kvh * D:(kvh + 1) * D])                    # [P, HO, 128]
        wvt = wtile(wv[kvh * D:(kvh + 1) * D])
        # woT: [P, HPC, HID] with j = h*P + p
        wot = np.ascontiguousarray(
            wo[:, j0:j0 + HPC * D].T.reshape(HPC, P, HID)
            .transpose(1, 0, 2)).astype(NPBF16)
        bqt = np.ascontiguousarray(
            bq[j0:j0 + HPC * D].reshape(HPC, P).T).astype(np.float32)
        bkt = bk[kvh * D:(kvh + 1) * D].reshape(P, 1).astype(np.float32)
        bvt = bv[kvh * D:(kvh + 1) * D].reshape(P, 1).astype(np.float32)
        in_maps.append({
            "xtt": xtt, "wqt": wqt, "wkt": wkt, "wvt": wvt, "wot": wot,
            "cost": cosT, "sint": sinT, "rot": R, "iden": iden, "ones": onesm,
            "tri": tri,
            "bq": bqt, "bk": bkt, "bv": bvt,
        })

    res = run_bass_kernel_spmd(nc, in_maps, list(range(NCORES)))
    out = np.zeros((T, HID), np.float32)
    for c in range(NCORES):
        out += res.results[c]["o_part"]
    return out.reshape(1, T, HID)


# revision 30
# speedup vs baseline: 1.1124x; 1.0071x over previous
"""Trainium2 Bass kernel for GQA causal attention (nn_Attention_89816356094768).

Math (per reference):
  q = x @ wq.T + bq ; k = x @ wk.T + bk ; v = x @ wv.T + bv
  RoPE on q, k; S = q @ k.T * D**-0.5 with causal mask; P = softmax(S)
  out = (P @ v) reassembled over heads @ wo.T

Sharding: tensor-parallel over heads across 8 cores. Core c owns q heads
(2c, 2c+1) and kv head c//4. Each core computes its two heads' attention and
a row-parallel partial of the output projection; the host sums the 8 partials.

On-device layout is fully transposed ("feature on partitions"): the host
pre-transposes x, the weights and the RoPE tables so every matmul contraction
dim lands on the 128 SBUF partitions with zero on-device transposes (except
v, which needs [s, d] layout for the P@v matmul and is PE-transposed).

Softmax is computed in S^T layout (scores [s, t], softmax over partitions):
exp without max-subtraction (logits are O(1) for the reference input
distribution), row sums via an all-ones-matrix matmul accumulated in PSUM,
normalization fused into the PSUM->SBUF copy of the P@v result.
"""

import numpy as np
import ml_dtypes
from contextlib import ExitStack

from concourse import bacc, tile, mybir
from concourse.bass_utils import run_bass_kernel_spmd

NQ, NKV, D = 16, 2, 128
HID = 2048
T = 4096
SCALE = D ** -0.5
NCORES = 8
HPC = NQ // NCORES          # q heads per core
P = 128                     # partitions
TS = 512                    # t-slice width (matmul moving free dim)
NT = T // P                 # 32 t tiles
NSL = T // TS               # 8 t slices
HO = HID // P               # 16 hidden k-tiles
BF16 = mybir.dt.bfloat16
F32 = mybir.dt.float32
AF = mybir.ActivationFunctionType
NPBF16 = ml_dtypes.bfloat16

_CACHE = {}


def _emit(nc, io, o_dram):
    with ExitStack() as top:
        tc = top.enter_context(tile.TileContext(nc))
        const = top.enter_context(tc.tile_pool(name="const", bufs=1))
        persist = top.enter_context(tc.tile_pool(name="persist", bufs=1))

        def cload(name, shape, dt, eng=None):
            t = const.tile(shape, dt, tag=name)
            (eng or nc.sync).dma_start(t[:], io[name][:])
            return t

        # Load order matters: the first projection matmuls need only wq/wk/wv
        # and the first x slice; defer the big phase-B constants so the PE can
        # start ~18us earlier.
        xs_pool = top.enter_context(tc.tile_pool(name="xs", bufs=2))

        # The first projection chain consumes wq and x-slice-0 h-tile by
        # h-tile, so interleave their loads in 4 chunks each and put
        # everything else behind them; phase-B-only constants trail last.
        wq = const.tile([P, HO, HPC * D], BF16, tag="wqt")
        xt0 = xs_pool.tile([P, HO, TS], BF16, tag="xt", name="xt0")
        for ch in range(4):
            hs = slice(4 * ch, 4 * (ch + 1))
            nc.sync.dma_start(wq[:, hs, :], io["wqt"][:, hs, :])
            nc.sync.dma_start(xt0[:, hs, :], io["xtt"][0, :, hs, :])
        bq = cload("bq", [P, HPC], F32)
        rot = cload("rot", [P, P], BF16)
        wk = cload("wkt", [P, HO, D], BF16)
        wv = cload("wvt", [P, HO, D], BF16)
        bk = cload("bk", [P, 1], F32)
        bv = cload("bv", [P, 1], F32)
        iden = cload("iden", [P, P], BF16)
        xt1 = xs_pool.tile([P, HO, TS], BF16, tag="xt", name="xt1")
        nc.sync.dma_start(xt1[:], io["xtt"][1])
        cosT = cload("cost", [P, T], BF16)
        sinT = cload("sint", [P, T], BF16)
        ones = cload("ones", [P, P], BF16, eng=nc.gpsimd)
        tri = cload("tri", [P, P], BF16, eng=nc.gpsimd)
        wo = cload("wot", [P, HPC, HID], BF16, eng=nc.gpsimd)

        qT = persist.tile([P, HPC, T], BF16, tag="qT")     # [d, h, t]
        kT = persist.tile([P, T], BF16, tag="kT")          # [d, s]
        vN = persist.tile([P, NT, P], BF16, tag="vN")      # [s_in, s_tile, d]
        aoT = persist.tile([P, HPC, T], BF16, tag="aoT")   # [d, h, t]

        # ---- Phase A: q/k/v projections (transposed), RoPE, v transpose ----
        with ExitStack() as pa:
            ppsum = pa.enter_context(tc.tile_pool(name="ppsum", bufs=4, space="PSUM"))
            rpsum = pa.enter_context(tc.tile_pool(name="rpsum", bufs=2, space="PSUM"))
            vpsum = pa.enter_context(tc.tile_pool(name="vpsum", bufs=2, space="PSUM"))
            rtmp = pa.enter_context(tc.tile_pool(name="rtmp", bufs=4))

            for sl in range(NSL):
                tsl = slice(sl * TS, (sl + 1) * TS)
                if sl == 0:
                    xt = xt0
                elif sl == 1:
                    xt = xt1
                else:
                    xt = xs_pool.tile([P, HO, TS], BF16, tag="xt")
                    nc.sync.dma_start(xt[:], io["xtt"][sl])

                # (weight AP [P, HO, 128], bias [P, 1], kind, head idx)
                jobs = [(wq[:, :, h * D:(h + 1) * D], bq[:, h:h + 1], "q", h)
                        for h in range(HPC)]
                jobs.append((wk, bk, "k", 0))
                jobs.append((wv, bv, "v", 0))

                for w_ap, b_ap, kind, h in jobs:
                    pl = ppsum.tile([P, TS], F32, tag="plin")
                    for ho in range(HO):
                        nc.tensor.matmul(pl[:], w_ap[:, ho, :], xt[:, ho, :],
                                         start=(ho == 0), stop=(ho == HO - 1))
                    lin = rtmp.tile([P, TS], BF16, tag="lin")
                    nc.scalar.activation(lin[:], pl[:], AF.Identity, bias=b_ap)
                    if kind in ("q", "k"):
                        rp = rpsum.tile([P, TS], F32, tag="rp")
                        nc.tensor.matmul(rp[:], rot[:], lin[:], start=True, stop=True)
                        tsin = rtmp.tile([P, TS], F32, tag="tsin")
                        nc.vector.tensor_mul(tsin[:], rp[:], sinT[:, tsl])
                        tcos = rtmp.tile([P, TS], F32, tag="tcos")
                        nc.vector.tensor_mul(tcos[:], lin[:], cosT[:, tsl])
                        dst = qT[:, h, tsl] if kind == "q" else kT[:, tsl]
                        nc.vector.tensor_add(dst, tsin[:], tcos[:])
                    else:
                        for tt in range(TS // P):
                            vp = vpsum.tile([P, P], BF16, tag="vtp")
                            nc.tensor.transpose(vp[:], lin[:, tt * P:(tt + 1) * P],
                                                iden[:])
                            nc.vector.tensor_copy(vN[:, sl * (TS // P) + tt, :], vp[:])

        # ---- Phase B + C: attention (S^T layout flash) + output projection ----
        with ExitStack() as pb:
            # PSUM budget (8 banks): stp 2x[P,1024]=4, avp 1, rsp 1, opp 2
            stp = pb.enter_context(tc.tile_pool(name="stp", bufs=2, space="PSUM"))
            avp = pb.enter_context(tc.tile_pool(name="avp", bufs=2, space="PSUM"))
            rsp = pb.enter_context(tc.tile_pool(name="rsp", bufs=1, space="PSUM"))
            opp = pb.enter_context(tc.tile_pool(name="opp", bufs=1, space="PSUM"))
            ptp = pb.enter_context(tc.tile_pool(name="ptp", bufs=6))
            nstage = pb.enter_context(tc.tile_pool(name="nstage", bufs=2))
            ostage = pb.enter_context(tc.tile_pool(name="ostage", bufs=2))

            for sl in range(NSL):
                tsl = slice(sl * TS, (sl + 1) * TS)
                n_s = 4 * sl + 4          # causal s tiles for this slice
                ng = n_s // 2
                for h in range(HPC):
                    av = avp.tile([P, TS], F32, tag="av")
                    rs = rsp.tile([P, TS], F32, tag="rs")
                    for g in range(ng):
                        st = stp.tile([P, 2 * TS], F32, tag="st")
                        pt = ptp.tile([P, 2 * TS], BF16, tag="pt")
                        # r >= 0 marks a diagonal-region s tile: its first
                        # r*P t-columns are fully masked, so skip them in the
                        # matmuls and exp, and mask only the diagonal block.
                        offs = [max(2 * g + i - 4 * sl, 0) * P for i in range(2)]
                        for i in range(2):
                            s_tile = 2 * g + i
                            off = offs[i]
                            nc.tensor.matmul(
                                st[:, i * TS + off:(i + 1) * TS],
                                kT[:, s_tile * P:(s_tile + 1) * P],
                                qT[:, h, sl * TS + off:(sl + 1) * TS],
                                start=True, stop=True)
                        if offs[1] == 0:
                            nc.scalar.activation(pt[:], st[:], AF.Exp, scale=SCALE)
                        else:
                            for i in range(2):
                                off = offs[i]
                                if off:
                                    nc.gpsimd.memset(pt[:, i * TS:i * TS + off], 0.0)
                                nc.scalar.activation(
                                    pt[:, i * TS + off:(i + 1) * TS],
                                    st[:, i * TS + off:(i + 1) * TS],
                                    AF.Exp, scale=SCALE)
                                c0 = i * TS + off
                                nc.vector.tensor_mul(pt[:, c0:c0 + P],
                                                     pt[:, c0:c0 + P], tri[:])
                        for i in range(2):
                            s_tile = 2 * g + i
                            off = offs[i]
                            first = s_tile == 0
                            last = s_tile == n_s - 1
                            seg = pt[:, i * TS + off:(i + 1) * TS]
                            nc.tensor.matmul(av[:, off:TS], vN[:, s_tile, :], seg,
                                             start=first, stop=last)
                            nc.tensor.matmul(rs[:, off:TS], ones[:], seg,
                                             start=first, stop=last)
                    rec = nstage.tile([P, TS], F32, tag="rec")
                    nc.vector.reciprocal(rec[:], rs[:])
                    nc.vector.tensor_mul(aoT[:, h, tsl], av[:], rec[:])

                # output projection for this slice's 4 row blocks; the final
                # slice borrows the score-psum pool (attention is done by
                # then) to double-buffer instead of serializing on opp
                fin = sl == NSL - 1
                for tt4 in range(4):
                    t_tile = 4 * sl + tt4
                    trow = slice(t_tile * P, (t_tile + 1) * P)
                    ot = ostage.tile([P, HID], F32, tag="ot")
                    for upair in range(2):
                        if fin:
                            op2 = stp.tile([P, 2 * TS], F32, tag="st", name="op2")
                            ops = [op2[:, :TS], op2[:, TS:]]
                        else:
                            ops = [opp.tile([P, TS], F32, tag="op", name=f"op{ui}")[:]
                                   for ui in range(2)]
                        for h in range(HPC):
                            lhs = aoT[:, h, trow]
                            for ui in range(2):
                                u0 = (upair * 2 + ui) * TS
                                nc.tensor.matmul(ops[ui], lhs,
                                                 wo[:, h, u0:u0 + TS],
                                                 start=(h == 0), stop=(h == HPC - 1))
                        if fin:
                            nc.any.tensor_copy(
                                ot[:, upair * 2 * TS:(upair + 1) * 2 * TS], op2[:])
                        else:
                            for ui in range(2):
                                u0 = (upair * 2 + ui) * TS
                                nc.vector.tensor_copy(ot[:, u0:u0 + TS], ops[ui])
                    nc.sync.dma_start(o_dram[trow, :], ot[:])


def _build_nc():
    nc = bacc.Bacc("TRN2", target_bir_lowering=False, debug=False,
                   enable_asserts=False, num_devices=NCORES)
    io = {}

    def din(name, shape, dt):
        io[name] = nc.dram_tensor(name, shape, dt, kind="ExternalInput").ap()

    din("xtt", [NSL, P, HO, TS], BF16)       # x^T pre-tiled per slice
    din("wqt", [P, HO, HPC * D], BF16)
    din("wkt", [P, HO, D], BF16)
    din("wvt", [P, HO, D], BF16)
    din("wot", [P, HPC, HID], BF16)
    din("cost", [P, T], BF16)
    din("sint", [P, T], BF16)
    din("rot", [P, P], BF16)
    din("iden", [P, P], BF16)
    din("ones", [P, P], BF16)
    din("tri", [P, P], BF16)
    din("bq", [P, HPC], F32)
    din("bk", [P, 1], F32)
    din("bv", [P, 1], F32)
    o = nc.dram_tensor("o_part", [T, HID], F32, kind="ExternalOutput").ap()
    _emit(nc, io, o)
    nc.compile()
    return nc


def _get_nc():
    if "nc" not in _CACHE:
        _CACHE["nc"] = _build_nc()
    return _CACHE["nc"]


def _consts():
    if "consts" in _CACHE:
        return _CACHE["consts"]
    # rotate_half as a matmul on q^T: out[d,t] = sum_e R[e,d] q^T[e,t]
    R = np.zeros((P, P), np.float32)
    for e in range(64):
        R[e, e + 64] = 1.0          # d >= 64 takes +q[d-64]
    for e in range(64, P):
        R[e, e - 64] = -1.0         # d < 64 takes -q[d+64]
    iden = np.eye(P, dtype=np.float32)
    onesm = np.ones((P, P), np.float32)
    tri = np.triu(np.ones((P, P), np.float32))
    _CACHE["consts"] = tuple(a.astype(NPBF16) for a in (R, iden, onesm, tri))
    return _CACHE["consts"]


def kernel(x, cos, sin, wq, bq, wk, bk, wv, bv, wo):
    x = np.asarray(x, dtype=np.float32)
    cos = np.asarray(cos, dtype=np.float32)
    sin = np.asarray(sin, dtype=np.float32)
    wq = np.asarray(wq, dtype=np.float32)
    bq = np.asarray(bq, dtype=np.float32)
    wk = np.asarray(wk, dtype=np.float32)
    bk = np.asarray(bk, dtype=np.float32)
    wv = np.asarray(wv, dtype=np.float32)
    bv = np.asarray(bv, dtype=np.float32)
    wo = np.asarray(wo, dtype=np.float32)

    nc = _get_nc()
    R, iden, onesm, tri = _consts()

    # x^T tiled: xtt[sl, p, ho, c] = x[0, sl*TS + c, ho*P + p]
    xT = np.ascontiguousarray(x[0].T).astype(NPBF16)              # [HID, T]
    xtt = np.ascontiguousarray(
        xT.reshape(HO, P, NSL, TS).transpose(2, 1, 0, 3))         # [NSL,P,HO,TS]
    cosT = np.ascontiguousarray(cos.T).astype(NPBF16)             # [P, T]
    sinT = np.ascontiguousarray(sin.T).astype(NPBF16)

    def wtile(wslice):  # [J, HID] -> [P, HO, J] with h = ho*P + p
        J = wslice.shape[0]
        return np.ascontiguousarray(
            wslice.T.reshape(HO, P, J).transpose(1, 0, 2)).astype(NPBF16)

    in_maps = []
    for c in range(NCORES):
        j0 = c * HPC * D
        kvh = c // (NCORES // NKV)
        wqt = wtile(wq[j0:j0 + HPC * D])                          # [P, HO, 256]
        wkt = wtile(wk[kvh * D:(kvh + 1) * D])                    # [P, HO, 128]
        wvt = wtile(wv[kvh * D:(kvh + 1) * D])
        # woT: [P, HPC, HID] with j = h*P + p
        wot = np.ascontiguousarray(
            wo[:, j0:j0 + HPC * D].T.reshape(HPC, P, HID)
            .transpose(1, 0, 2)).astype(NPBF16)
        bqt = np.ascontiguousarray(
            bq[j0:j0 + HPC * D].reshape(HPC, P).T).astype(np.float32)
        bkt = bk[kvh * D:(kvh + 1) * D].reshape(P, 1).astype(np.float32)
        bvt = bv[kvh * D:(kvh + 1) * D].reshape(P, 1).astype(np.float32)
        in_maps.append({
            "xtt": xtt, "wqt": wqt, "wkt": wkt, "wvt": wvt, "wot": wot,
            "cost": cosT, "sint": sinT, "rot": R, "iden": iden, "ones": onesm,
            "tri": tri,
            "bq": bqt, "bk": bkt, "bv": bvt,
        })

    res = run_bass_kernel_spmd(nc, in_maps, list(range(NCORES)))
    out = np.zeros((T, HID), np.float32)
    for c in range(NCORES):
        out += res.results[c]["o_part"]
    return out.reshape(1, T, HID)
